# revision 31
# baseline (speedup 1.0000x reference)
"""Cumulative-FFT Trainium2 kernel (final).

out[b,t,d,k,c] = pos_norm[t] * cumsum_t( x[b,t,d] * twiddles[t,k,c] )

Shapes (hardcoded): x (4,1024,512) bf16, twiddles (1024,32,2) bf16,
pos_norm (1024,) bf16  ->  out (4,1024,512,32,2) bf16.

Sharding: 8 cores = batch(4) x d_model-half(2). Each core computes a
(1024, 256*64) bf16 shard (32 MiB) -- data-parallel over B, tensor-parallel
over D, nothing crosses cores.

Per-core algorithm: cumsum along t as per-block (BLK=128) triangular
matmuls on the PE. The moving operand c holds the bf16 contributions
c[s, kc*256+d] = x[s,d]*tw[s,kc], built by ONE 2x-mode DVE tensor_tensor
per block against a host-precomputed 16x-replicated tw tile (8.6us).
The carry (column sums of all previous blocks) is folded into c's row 0
by an accumulating SWDGE DMA; utri[s,t] = pos[t0+t]*(s<=t) then finishes
each block's 16384 output columns in one PE pass. Evictions (PSUM ->
bf16 SBUF, 1x-capped) split DVE 4 / ACT 12 per block; stores ship as
four 1-MiB column chunks per block on the sync HW-DGE queue.

Steady state is DVE-bound at ~13.1us/block (TT 8.6 + 4 casts 4.5;
engine-capacity floor ~12.9 given the PSUM-slot ladder); the store DMA
floor is 11.7us/block (32 MiB @ 358 GB/s HBM-per-core).

Changes vs the 145.9us v1 baseline (-> 139.8us):
 - Carry chain (7 delta matmuls tw_k^T @ x_k) fully decoupled: deltas
   run early at psum-rotation-friendly spots in block 0, ACT (not DVE)
   evicts them, GpSimd does the prefix adds. carry_7 stops gating the
   tail (v1 lost ~5us waiting for it after TT_7).
 - Last block's store chunks are emitted AFTER all its evictions and
   alternate sync/scalar queues: a dma_start inside ACT's instruction
   stream head-of-line-blocks its remaining copies on DVE casts.
 - Head loads reordered (x block 0 first, TT_0 4-way chunked) so
   block-0 matmuls start at ~12.5us instead of 16.4.

Dead ends, measured (do not retry):
 - A-scheme (fold tw into the PE stationary, A_kc = utri*tw, half the
   DVE build cost): needs the carry re-added via K=1 rank-1 matmuls,
   and ANY partial-K matmul (K=1 or K=32) permanently throttles the PE
   clock to 1.2 GHz (HAM never un-throttles; 75us of gapless MMs all
   cold). Full-K carry matmuls cost their column count (+6.8us/block).
   Every matmul must be K=128.
 - GpSimd c-build offload: under concurrent DVE activity GPS loses the
   shared SBUF port (16-kc slice 7.9us idle -> ~10us, [64,256] adds
   5-7us) and its FIFO delays the carry folds -> 192us.
 - Giving ACT 13 sequential groups (DVE 3): the PSUM-slot ladder makes
   tail-group matmuls wait ACT's serial march -> 14.7us/block.
 - Scalar-queue stores in steady state: ACT head-of-line stall.
 - N=1024 matmuls (2-bank PSUM out): codegen/runtime failure.

Other trace facts that shaped this:
 - HWDGE stripes across 16 SDMA engines only for 128-partition DMAs.
 - DVE TT bf16 is 2x ((58+FD/2)/0.96GHz); PSUM-source evictions are 1x
   on DVE ((120+FD)/0.96) and ACT ((172+FD)/1.2).
 - PSUM 8 banks = pmain bufs=4 x 2 banks; deltas share the rotation.
 - ~9us fixed epilogue (per-engine semaphore finalization) + ~6.5us
   init are outside kernel control.
"""

import sys

sys.path.insert(0, "/opt/trn_rl_repo")

import ml_dtypes
import numpy as np

import concourse.bass as bass
import concourse.mybir as mybir
import concourse.tile as tile
from concourse import bacc
import concourse.bass_utils as _bu
from concourse.bass_utils import run_bass_kernel_spmd

B, T, D = 4, 1024, 512
KC = 64            # 32 freqs x (cos,sin), flattened innermost dims of out
DSH = D // 2       # d-slice per core
NKC = DSH * KC     # free elements per t per core (16384)
BLK = 128          # rows per t-block
NBLK = T // BLK    # 8

BF16 = mybir.dt.bfloat16
F32 = mybir.dt.float32

# eviction split per block: DVE gets the tail groups (its queue first
# drains the next block's 8.6us TT); 3 and 4 alternate so DVE/ACT load
# balances across block pairs. Last block: DVE free (no next TT), so
# interleave odd/even for concurrency.
_DVE_GROUPS_BY_BLK = [
    (9, 10, 11, 12), (9, 10, 11, 12), (9, 10, 11, 12), (9, 10, 11, 12),
    (9, 10, 11, 12), (9, 10, 11, 12), (9, 10, 11, 12),
    (0, 1, 2, 3, 4, 5, 6, 7),
]
# stores: 4 chunks of 4 groups, emitted after the eviction of the
# chunk's last group; alternate sync/scalar queues. Last block: finer.
_CHUNK_AFTER = {3: (0, 4), 7: (4, 4), 11: (8, 4), 15: (12, 4)}
_CHUNK_AFTER_LAST = {
    1: (0, 2), 3: (2, 2), 5: (4, 2), 7: (6, 2), 9: (8, 2),
    11: (10, 2), 13: (12, 2), 15: (14, 2),
}

LAST_RESULTS = None  # set by kernel(); test.py reads exec_time_ns from here


def _build_utri(pos_norm: np.ndarray) -> np.ndarray:
    """Stationary operands for all blocks, packed (128, NBLK*128) bf16."""
    pos = np.asarray(pos_norm).astype(np.float32)
    utri = np.zeros((128, NBLK * 128), np.float32)
    s = np.arange(128)[:, None]
    t = np.arange(128)[None, :]
    for k in range(NBLK):
        t0 = k * BLK
        utri[:, 128 * k : 128 * (k + 1)] = (s <= t) * pos[t0 : t0 + 128][None, :]
    return utri.astype(ml_dtypes.bfloat16)


def _build_program() -> bass.Bass:
    nc = bacc.Bacc("TRN2", target_bir_lowering=False, debug=False)
    x_d = nc.dram_tensor("x_shard", [T, DSH], BF16, kind="ExternalInput").ap()
    tw_d = nc.dram_tensor("tw", [T, KC], BF16, kind="ExternalInput").ap()
    utri_d = nc.dram_tensor("utri", [128, NBLK * 128], BF16, kind="ExternalInput").ap()
    twrep_d = nc.dram_tensor("twrep", [T, KC * 16], BF16, kind="ExternalInput").ap()
    out_d = nc.dram_tensor("out_shard", [T, NKC], BF16, kind="ExternalOutput").ap()

    with tile.TileContext(nc) as tc:
        with (
            tc.tile_pool(name="singles", bufs=1) as singles,
            tc.tile_pool(name="cp", bufs=3) as cp,
            tc.tile_pool(name="outp", bufs=2) as outp,
            tc.tile_pool(name="pmain", bufs=4, space="PSUM") as pmain,
        ):
            # loads, all 128-partition. sync queue: x block 0 (gates TT_0),
            # tw (gates the delta chain), rest of x. scalar queue: rep
            # block 0 + utri (gate TT_0 / block-0 matmuls), rest of rep.
            x_all = singles.tile([128, NBLK * DSH], BF16)
            x_all_v = x_all.rearrange("p (j d) -> p j d", j=NBLK)
            x_d_v = x_d.rearrange("(j p) d -> p j d", p=128)
            nc.sync.dma_start(out=x_all_v[:, 0:1], in_=x_d_v[:, 0:1])
            tw_all = singles.tile([128, NBLK * KC], BF16)
            nc.sync.dma_start(
                out=tw_all.rearrange("p (j k) -> p j k", j=NBLK),
                in_=tw_d.rearrange("(j p) k -> p j k", p=128),
            )
            nc.sync.dma_start(out=x_all_v[:, 1:NBLK], in_=x_d_v[:, 1:NBLK])
            rep_all = singles.tile([128, NBLK * KC * 16], BF16)
            rep_v0 = rep_all.rearrange("p (j r) -> p j r", j=NBLK)
            twrep_v = twrep_d.rearrange("(j p) r -> p j r", p=128)
            nc.scalar.dma_start(out=rep_v0[:, 0:1], in_=twrep_v[:, 0:1])
            utri_sb = singles.tile([128, NBLK * 128], BF16)
            nc.scalar.dma_start(out=utri_sb[:, :], in_=utri_d[:, :])
            nc.scalar.dma_start(out=rep_v0[:, 1:3], in_=twrep_v[:, 1:3])
            nc.scalar.dma_start(out=rep_v0[:, 3:NBLK], in_=twrep_v[:, 3:NBLK])

            def build_c(k, carry, nchunks=1):
                # contributions, kc-major: c[s, kc*DSH + d] = x[s,d] * tw[s,kc]
                # as bf16 tensor_tensor(s) in the DVE 2x mode; see v1 notes.
                rep16 = rep_all[:, k * KC * 16 : (k + 1) * KC * 16]
                x_sb = x_all[:, k * DSH : (k + 1) * DSH]
                c_sb = cp.tile([128, NKC], BF16)
                x_v3 = x_sb.rearrange("p (b c) -> p b c", c=16).unsqueeze(1)
                rep_v3 = rep16.rearrange("p (a c) -> p a c", c=16).unsqueeze(2)
                # note: offloading part of this TT to GpSimd fails badly --
                # under concurrent DVE activity GPS loses its shared SBUF
                # port (16-kc slice: 7.9us idle -> ~10us; [64,256] adds blow
                # up to 5-7us) and the GPS FIFO then delays the carry folds.
                kcn = KC // nchunks
                for ci in range(nchunks):
                    ka, kb = ci * kcn, (ci + 1) * kcn
                    c_v = c_sb[:, ka * DSH : kb * DSH].rearrange(
                        "p (a b c) -> p a b c", b=16, c=16
                    )
                    nc.vector.tensor_mul(
                        c_v,
                        x_v3.broadcast_to((128, kcn, 16, 16)),
                        rep_v3[:, ka:kb].broadcast_to((128, kcn, 16, 16)),
                    )
                # fold the (precomputed) carry into c's first row: SWDGE DMA
                # with inline CCE add; utri row 0 is pos[t] for all t, so the
                # matmul finishes the block including the carry.
                if carry is not None:
                    nc.gpsimd.dma_start(
                        out=c_sb[0:1, :], in_=carry[:, :],
                        accum_op=mybir.AluOpType.add,
                    )
                return c_sb

            # TT_0 first (gates block 0; chunked so block-0 matmuls start
            # after the first DVE chunk)
            c_cur = build_c(0, None, nchunks=4)

            # carry chain: carries[k] = sum_{j<k} tw_j^T @ x_j. Emitted
            # spread through block 0 (delta_0 up front, the rest at psum-
            # rotation-friendly spots) so the PE never stalls on pg slots.
            # The chain's PSUM consumers must NOT sit on DVE (its queue is
            # busy with TTs): ACT evicts each delta to SBUF, GpSimd runs the
            # prefix adds. Keeps carry_7 off the tail's critical path too.
            dsb_all = singles.tile([KC, (NBLK - 1) * DSH], BF16)
            carr_all = singles.tile([KC, (NBLK - 2) * DSH], BF16)
            carries = [None, dsb_all[:, 0:DSH]] + [
                carr_all[:, (k - 1) * DSH : k * DSH] for k in range(1, NBLK - 1)
            ]

            def emit_delta(k):
                delta = pmain.tile([KC, DSH], F32, tag="pg", name=f"delta{k}")
                nc.tensor.matmul(
                    delta[:, :],
                    lhsT=tw_all[:, k * KC : (k + 1) * KC],
                    rhs=x_all[:, k * DSH : (k + 1) * DSH],
                    start=True, stop=True,
                )
                dsb = dsb_all[:, k * DSH : (k + 1) * DSH]
                nc.scalar.copy(dsb, delta[:, :])
                if k > 0:
                    nc.gpsimd.tensor_add(carries[k + 1], carries[k], dsb)

            emit_delta(0)

            for k in range(NBLK):
                if k + 1 < NBLK:
                    # next block's contributions build while this block runs
                    c_next = build_c(k + 1, carries[k + 1])
                else:
                    c_next = None

                og = outp.tile([128, NKC], BF16)
                dve_groups = _DVE_GROUPS_BY_BLK[k]
                chunk_after = _CHUNK_AFTER_LAST if k == NBLK - 1 else _CHUNK_AFTER
                lhsT = utri_sb[:, 128 * k : 128 * (k + 1)]
                n_ship = 0
                for gi in range(16):
                    pg = pmain.tile([128, 1024], F32, tag="pg", name=f"pg{k}_{gi}")
                    for jj in range(2):
                        nc.tensor.matmul(
                            pg[:, jj * 512 : (jj + 1) * 512],
                            lhsT=lhsT,
                            rhs=c_cur[:, (gi * 2 + jj) * 512 : (gi * 2 + jj + 1) * 512],
                            start=True, stop=True,
                        )
                    col = gi * 1024
                    if gi in dve_groups:
                        nc.vector.tensor_copy(og[:, col : col + 1024], pg[:, :])
                    else:
                        nc.scalar.copy(og[:, col : col + 1024], pg[:, :])
                    if k == 0 and gi == 7:
                        for j in range(1, 4):
                            emit_delta(j)
                    if k == 0 and gi == 11:
                        for j in range(4, 7):
                            emit_delta(j)
                    if k < NBLK - 1 and gi in chunk_after:
                        c0, cg = chunk_after[gi]
                        # steady-state stores go ONLY on the dedicated Sync
                        # queue: a dma_start in the ACT instruction stream
                        # blocks head-of-line on DVE's tail casts and stalls
                        # the next block's copies.
                        nc.sync.dma_start(
                            out=out_d[k * BLK : (k + 1) * BLK,
                                      c0 * 1024 : (c0 + cg) * 1024],
                            in_=og[:, c0 * 1024 : (c0 + cg) * 1024],
                        )
                if k == NBLK - 1:
                    # last block: emit every chunk AFTER all evictions so the
                    # scalar-queue DMAs never head-of-line-block ACT's copies;
                    # two queues drain the final 4 MiB in parallel.
                    for ci, (c0, cg) in enumerate(sorted(chunk_after.values())):
                        eng = nc.sync if ci % 2 == 0 else nc.scalar
                        eng.dma_start(
                            out=out_d[k * BLK : (k + 1) * BLK,
                                      c0 * 1024 : (c0 + cg) * 1024],
                            in_=og[:, c0 * 1024 : (c0 + cg) * 1024],
                        )
                c_cur = c_next
    nc.compile()
    return nc


def kernel(**inputs) -> np.ndarray:
    global LAST_RESULTS
    x = np.asarray(inputs["x"])                       # (4,1024,512) bf16
    tw = np.asarray(inputs["twiddles"])               # (1024,32,2) bf16
    pos = np.asarray(inputs["pos_norm"])              # (1024,) bf16

    tw2 = np.ascontiguousarray(tw.reshape(T, KC))
    twrep = np.ascontiguousarray(np.repeat(tw2, 16, axis=1))
    utri = _build_utri(pos)

    in_maps = []
    for core in range(8):
        b, dh = core // 2, core % 2
        xs = np.ascontiguousarray(x[b, :, dh * DSH : (dh + 1) * DSH])
        in_maps.append(
            {"x_shard": xs, "tw": tw2, "utri": utri, "twrep": twrep}
        )

    nc = _build_program()
    res = run_bass_kernel_spmd(nc, in_maps, core_ids=list(range(8)))
    LAST_RESULTS = res

    out = np.empty((B, T, D, KC // 2, 2), dtype=x.dtype)
    for core in range(8):
        b, dh = core // 2, core % 2
        o = np.asarray(res.results[core]["out_shard"])  # (T, NKC) kc-major
        o = o.reshape(T, KC, DSH).transpose(0, 2, 1)    # -> (T, DSH, KC)
        out[b, :, dh * DSH : (dh + 1) * DSH, :, :] = o.reshape(T, DSH, KC // 2, 2)
    return out


if __name__ == "__main__":
    rng = np.random.default_rng(0)
    demo = {
        "x": rng.standard_normal((B, T, D), np.float32).astype(ml_dtypes.bfloat16),
        "twiddles": rng.standard_normal((T, KC // 2, 2), np.float32).astype(
            ml_dtypes.bfloat16
        ),
        "pos_norm": (1.0 / np.sqrt(np.arange(1, T + 1, dtype=np.float32))).astype(
            ml_dtypes.bfloat16
        ),
    }
    print(kernel(**demo).shape)


# revision 32
# speedup vs baseline: 1.0121x; 1.0121x over previous
"""Cumulative-FFT Trainium2 kernel (final).

out[b,t,d,k,c] = pos_norm[t] * cumsum_t( x[b,t,d] * twiddles[t,k,c] )

Shapes (hardcoded): x (4,1024,512) bf16, twiddles (1024,32,2) bf16,
pos_norm (1024,) bf16  ->  out (4,1024,512,32,2) bf16.

Sharding: 8 cores = batch(4) x d_model-half(2). Each core computes a
(1024, 256*64) bf16 shard (32 MiB) -- data-parallel over B, tensor-parallel
over D, nothing crosses cores.

Per-core algorithm: cumsum along t as per-block (BLK=128) triangular
matmuls on the PE. The moving operand c holds the bf16 contributions
c[s, kc*256+d] = x[s,d]*tw[s,kc], built by ONE 2x-mode DVE tensor_tensor
per block against a host-precomputed 16x-replicated tw tile (8.6us).
The carry (column sums of all previous blocks) is folded into c's row 0
by an accumulating SWDGE DMA; utri[s,t] = pos[t0+t]*(s<=t) then finishes
each block's 16384 output columns in one PE pass. Evictions (PSUM ->
bf16 SBUF, 1x-capped) split DVE 4 / ACT 12 per block; stores ship as
four 1-MiB column chunks per block on the sync HW-DGE queue.

Steady state is DVE-bound at ~13.1us/block (TT 8.6 + 4 casts 4.5;
engine-capacity floor ~12.9 given the PSUM-slot ladder); the store DMA
floor is 11.7us/block (32 MiB @ 358 GB/s HBM-per-core).

Changes vs the 145.9us v1 baseline (-> 139.8us):
 - Carry chain (7 delta matmuls tw_k^T @ x_k) fully decoupled: deltas
   run early at psum-rotation-friendly spots in block 0, ACT (not DVE)
   evicts them, GpSimd does the prefix adds. carry_7 stops gating the
   tail (v1 lost ~5us waiting for it after TT_7).
 - Last block's store chunks are emitted AFTER all its evictions and
   alternate sync/scalar queues: a dma_start inside ACT's instruction
   stream head-of-line-blocks its remaining copies on DVE casts.
 - Head loads reordered (x block 0 first, TT_0 4-way chunked) so
   block-0 matmuls start at ~12.5us instead of 16.4.

Dead ends, measured (do not retry):
 - A-scheme (fold tw into the PE stationary, A_kc = utri*tw, half the
   DVE build cost): needs the carry re-added via K=1 rank-1 matmuls,
   and ANY partial-K matmul (K=1 or K=32) permanently throttles the PE
   clock to 1.2 GHz (HAM never un-throttles; 75us of gapless MMs all
   cold). Full-K carry matmuls cost their column count (+6.8us/block).
   Every matmul must be K=128.
 - GpSimd c-build offload: under concurrent DVE activity GPS loses the
   shared SBUF port (16-kc slice 7.9us idle -> ~10us, [64,256] adds
   5-7us) and its FIFO delays the carry folds -> 192us.
 - Giving ACT 13 sequential groups (DVE 3): the PSUM-slot ladder makes
   tail-group matmuls wait ACT's serial march -> 14.7us/block.
 - Scalar-queue stores in steady state: ACT head-of-line stall.
 - N=1024 matmuls (2-bank PSUM out): codegen/runtime failure.

Other trace facts that shaped this:
 - HWDGE stripes across 16 SDMA engines only for 128-partition DMAs.
 - DVE TT bf16 is 2x ((58+FD/2)/0.96GHz); PSUM-source evictions are 1x
   on DVE ((120+FD)/0.96) and ACT ((172+FD)/1.2).
 - PSUM 8 banks = pmain bufs=4 x 2 banks; deltas share the rotation.
 - ~9us fixed epilogue (per-engine semaphore finalization) + ~6.5us
   init are outside kernel control.
"""

import sys

sys.path.insert(0, "/opt/trn_rl_repo")

import ml_dtypes
import numpy as np

import concourse.bass as bass
import concourse.mybir as mybir
import concourse.tile as tile
from concourse import bacc
import concourse.bass_utils as _bu
from concourse.bass_utils import run_bass_kernel_spmd

B, T, D = 4, 1024, 512
KC = 64            # 32 freqs x (cos,sin), flattened innermost dims of out
DSH = D // 2       # d-slice per core
NKC = DSH * KC     # free elements per t per core (16384)
BLK = 128          # rows per t-block
NBLK = T // BLK    # 8

BF16 = mybir.dt.bfloat16
F32 = mybir.dt.float32

# eviction split per block: DVE gets the tail groups (its queue first
# drains the next block's 8.6us TT); 3 and 4 alternate so DVE/ACT load
# balances across block pairs. Last block: DVE free (no next TT), so
# interleave odd/even for concurrency.
_DVE_GROUPS_BY_BLK = [
    (9, 10, 11, 12), (9, 10, 11, 12), (9, 10, 11, 12), (9, 10, 11, 12),
    (9, 10, 11, 12), (9, 10, 11, 12), (9, 10, 11, 12),
    (1, 3, 5, 7, 9, 11, 13, 15),
]
# stores: 4 chunks of 4 groups, emitted after the eviction of the
# chunk's last group; alternate sync/scalar queues. Last block: finer.
_CHUNK_AFTER = {3: (0, 4), 7: (4, 4), 11: (8, 4), 15: (12, 4)}
_CHUNK_AFTER_LAST = {
    1: (0, 2), 3: (2, 2), 5: (4, 2), 7: (6, 2), 9: (8, 2),
    11: (10, 2), 13: (12, 2), 15: (14, 2),
}

LAST_RESULTS = None  # set by kernel(); test.py reads exec_time_ns from here


def _build_utri(pos_norm: np.ndarray) -> np.ndarray:
    """Stationary operands for all blocks, packed (128, NBLK*128) bf16."""
    pos = np.asarray(pos_norm).astype(np.float32)
    utri = np.zeros((128, NBLK * 128), np.float32)
    s = np.arange(128)[:, None]
    t = np.arange(128)[None, :]
    for k in range(NBLK):
        t0 = k * BLK
        utri[:, 128 * k : 128 * (k + 1)] = (s <= t) * pos[t0 : t0 + 128][None, :]
    return utri.astype(ml_dtypes.bfloat16)


def _build_program() -> bass.Bass:
    nc = bacc.Bacc("TRN2", target_bir_lowering=False, debug=False)
    x_d = nc.dram_tensor("x_shard", [T, DSH], BF16, kind="ExternalInput").ap()
    tw_d = nc.dram_tensor("tw", [T, KC], BF16, kind="ExternalInput").ap()
    utri_d = nc.dram_tensor("utri", [128, NBLK * 128], BF16, kind="ExternalInput").ap()
    twrep_d = nc.dram_tensor("twrep", [T, KC * 16], BF16, kind="ExternalInput").ap()
    out_d = nc.dram_tensor("out_shard", [T, NKC], BF16, kind="ExternalOutput").ap()

    with tile.TileContext(nc) as tc:
        with (
            tc.tile_pool(name="singles", bufs=1) as singles,
            tc.tile_pool(name="cp", bufs=3) as cp,
            tc.tile_pool(name="outp", bufs=2) as outp,
            tc.tile_pool(name="pmain", bufs=4, space="PSUM") as pmain,
        ):
            # loads, all 128-partition. sync queue: x block 0 (gates TT_0),
            # tw (gates the delta chain), rest of x. scalar queue: rep
            # block 0 + utri (gate TT_0 / block-0 matmuls), rest of rep.
            x_all = singles.tile([128, NBLK * DSH], BF16)
            x_all_v = x_all.rearrange("p (j d) -> p j d", j=NBLK)
            x_d_v = x_d.rearrange("(j p) d -> p j d", p=128)
            nc.sync.dma_start(out=x_all_v[:, 0:1], in_=x_d_v[:, 0:1])
            tw_all = singles.tile([128, NBLK * KC], BF16)
            nc.sync.dma_start(
                out=tw_all.rearrange("p (j k) -> p j k", j=NBLK),
                in_=tw_d.rearrange("(j p) k -> p j k", p=128),
            )
            nc.sync.dma_start(out=x_all_v[:, 1:NBLK], in_=x_d_v[:, 1:NBLK])
            rep_all = singles.tile([128, NBLK * KC * 16], BF16)
            rep_v0 = rep_all.rearrange("p (j r) -> p j r", j=NBLK)
            twrep_v = twrep_d.rearrange("(j p) r -> p j r", p=128)
            nc.scalar.dma_start(out=rep_v0[:, 0:1], in_=twrep_v[:, 0:1])
            utri_sb = singles.tile([128, NBLK * 128], BF16)
            nc.scalar.dma_start(out=utri_sb[:, :], in_=utri_d[:, :])
            nc.scalar.dma_start(out=rep_v0[:, 1:3], in_=twrep_v[:, 1:3])
            nc.scalar.dma_start(out=rep_v0[:, 3:NBLK], in_=twrep_v[:, 3:NBLK])

            def build_c(k, carry, nchunks=1):
                # contributions, kc-major: c[s, kc*DSH + d] = x[s,d] * tw[s,kc]
                # as bf16 tensor_tensor(s) in the DVE 2x mode; see v1 notes.
                rep16 = rep_all[:, k * KC * 16 : (k + 1) * KC * 16]
                x_sb = x_all[:, k * DSH : (k + 1) * DSH]
                c_sb = cp.tile([128, NKC], BF16)
                x_v3 = x_sb.rearrange("p (b c) -> p b c", c=16).unsqueeze(1)
                rep_v3 = rep16.rearrange("p (a c) -> p a c", c=16).unsqueeze(2)
                # note: offloading part of this TT to GpSimd fails badly --
                # under concurrent DVE activity GPS loses its shared SBUF
                # port (16-kc slice: 7.9us idle -> ~10us; [64,256] adds blow
                # up to 5-7us) and the GPS FIFO then delays the carry folds.
                kcn = KC // nchunks
                for ci in range(nchunks):
                    ka, kb = ci * kcn, (ci + 1) * kcn
                    c_v = c_sb[:, ka * DSH : kb * DSH].rearrange(
                        "p (a b c) -> p a b c", b=16, c=16
                    )
                    nc.vector.tensor_mul(
                        c_v,
                        x_v3.broadcast_to((128, kcn, 16, 16)),
                        rep_v3[:, ka:kb].broadcast_to((128, kcn, 16, 16)),
                    )
                # fold the (precomputed) carry into c's first row: SWDGE DMA
                # with inline CCE add; utri row 0 is pos[t] for all t, so the
                # matmul finishes the block including the carry.
                if carry is not None:
                    nc.gpsimd.dma_start(
                        out=c_sb[0:1, :], in_=carry[:, :],
                        accum_op=mybir.AluOpType.add,
                    )
                return c_sb

            # TT_0 first (gates block 0; chunked so block-0 matmuls start
            # after the first DVE chunk)
            c_cur = build_c(0, None, nchunks=4)

            # carry chain: carries[k] = sum_{j<k} tw_j^T @ x_j. Emitted
            # spread through block 0 (delta_0 up front, the rest at psum-
            # rotation-friendly spots) so the PE never stalls on pg slots.
            # The chain's PSUM consumers must NOT sit on DVE (its queue is
            # busy with TTs): ACT evicts each delta to SBUF, GpSimd runs the
            # prefix adds. Keeps carry_7 off the tail's critical path too.
            dsb_all = singles.tile([KC, (NBLK - 1) * DSH], BF16)
            carr_all = singles.tile([KC, (NBLK - 2) * DSH], BF16)
            carries = [None, dsb_all[:, 0:DSH]] + [
                carr_all[:, (k - 1) * DSH : k * DSH] for k in range(1, NBLK - 1)
            ]

            def emit_delta(k):
                delta = pmain.tile([KC, DSH], F32, tag="pg", name=f"delta{k}")
                nc.tensor.matmul(
                    delta[:, :],
                    lhsT=tw_all[:, k * KC : (k + 1) * KC],
                    rhs=x_all[:, k * DSH : (k + 1) * DSH],
                    start=True, stop=True,
                )
                dsb = dsb_all[:, k * DSH : (k + 1) * DSH]
                nc.scalar.copy(dsb, delta[:, :])
                if k > 0:
                    nc.gpsimd.tensor_add(carries[k + 1], carries[k], dsb)

            emit_delta(0)

            for k in range(NBLK):
                if k + 1 < NBLK:
                    # next block's contributions build while this block runs
                    c_next = build_c(k + 1, carries[k + 1])
                else:
                    c_next = None

                og = outp.tile([128, NKC], BF16)
                dve_groups = _DVE_GROUPS_BY_BLK[k]
                chunk_after = _CHUNK_AFTER_LAST if k == NBLK - 1 else _CHUNK_AFTER
                lhsT = utri_sb[:, 128 * k : 128 * (k + 1)]
                n_ship = 0
                for gi in range(16):
                    pg = pmain.tile([128, 1024], F32, tag="pg", name=f"pg{k}_{gi}")
                    for jj in range(2):
                        nc.tensor.matmul(
                            pg[:, jj * 512 : (jj + 1) * 512],
                            lhsT=lhsT,
                            rhs=c_cur[:, (gi * 2 + jj) * 512 : (gi * 2 + jj + 1) * 512],
                            start=True, stop=True,
                        )
                    col = gi * 1024
                    if gi in dve_groups:
                        nc.vector.tensor_copy(og[:, col : col + 1024], pg[:, :])
                    else:
                        nc.scalar.copy(og[:, col : col + 1024], pg[:, :])
                    if k == 0 and gi == 7:
                        for j in range(1, 4):
                            emit_delta(j)
                    if k == 0 and gi == 11:
                        for j in range(4, 7):
                            emit_delta(j)
                    if k < NBLK - 1 and gi in chunk_after:
                        c0, cg = chunk_after[gi]
                        # steady-state stores go ONLY on the dedicated Sync
                        # queue: a dma_start in the ACT instruction stream
                        # blocks head-of-line on DVE's tail casts and stalls
                        # the next block's copies.
                        nc.sync.dma_start(
                            out=out_d[k * BLK : (k + 1) * BLK,
                                      c0 * 1024 : (c0 + cg) * 1024],
                            in_=og[:, c0 * 1024 : (c0 + cg) * 1024],
                        )
                if k == NBLK - 1:
                    # last block: emit every chunk AFTER all evictions so the
                    # scalar-queue DMAs never head-of-line-block ACT's copies;
                    # two queues drain the final 4 MiB in parallel.
                    for ci, (c0, cg) in enumerate(sorted(chunk_after.values())):
                        eng = nc.sync if ci % 2 == 0 else nc.scalar
                        eng.dma_start(
                            out=out_d[k * BLK : (k + 1) * BLK,
                                      c0 * 1024 : (c0 + cg) * 1024],
                            in_=og[:, c0 * 1024 : (c0 + cg) * 1024],
                        )
                c_cur = c_next
    nc.compile()
    return nc


def kernel(**inputs) -> np.ndarray:
    global LAST_RESULTS
    x = np.asarray(inputs["x"])                       # (4,1024,512) bf16
    tw = np.asarray(inputs["twiddles"])               # (1024,32,2) bf16
    pos = np.asarray(inputs["pos_norm"])              # (1024,) bf16

    tw2 = np.ascontiguousarray(tw.reshape(T, KC))
    twrep = np.ascontiguousarray(np.repeat(tw2, 16, axis=1))
    utri = _build_utri(pos)

    in_maps = []
    for core in range(8):
        b, dh = core // 2, core % 2
        xs = np.ascontiguousarray(x[b, :, dh * DSH : (dh + 1) * DSH])
        in_maps.append(
            {"x_shard": xs, "tw": tw2, "utri": utri, "twrep": twrep}
        )

    nc = _build_program()
    res = run_bass_kernel_spmd(nc, in_maps, core_ids=list(range(8)))
    LAST_RESULTS = res

    out = np.empty((B, T, D, KC // 2, 2), dtype=x.dtype)
    for core in range(8):
        b, dh = core // 2, core % 2
        o = np.asarray(res.results[core]["out_shard"])  # (T, NKC) kc-major
        o = o.reshape(T, KC, DSH).transpose(0, 2, 1)    # -> (T, DSH, KC)
        out[b, :, dh * DSH : (dh + 1) * DSH, :, :] = o.reshape(T, DSH, KC // 2, 2)
    return out


if __name__ == "__main__":
    rng = np.random.default_rng(0)
    demo = {
        "x": rng.standard_normal((B, T, D), np.float32).astype(ml_dtypes.bfloat16),
        "twiddles": rng.standard_normal((T, KC // 2, 2), np.float32).astype(
            ml_dtypes.bfloat16
        ),
        "pos_norm": (1.0 / np.sqrt(np.arange(1, T + 1, dtype=np.float32))).astype(
            ml_dtypes.bfloat16
        ),
    }
    print(kernel(**demo).shape)
